# revision 7
# baseline (speedup 1.0000x reference)
"""LongConv kernel for Trainium2 (8 NeuronCores, SPMD).

Reference computation (B=4, C=2, H=768, L=4096):
    k   = soft_threshold(kernel, lam=0.1)            # (C, H, 2L)
    y   = irfft(rfft(u, 2L) * rfft(k, 2L))[..., :L]  # FFT long conv
    y  += u * D                                      # skip
    y   = gelu(y.reshape(B, C*H, L))                 # tanh-approx gelu
    out = GLU((y^T @ W + b))^T                       # (B, H, L)

Key algebraic fact: kernel is drawn as 0.002*randn and lam=0.1, so the
soft-threshold zeroes it out exactly (would need a 50-sigma sample not to).
We verify that on the actual data (exact elementwise check, not an
assumption) and dispatch:

  * fast path  (k == 0): y = gelu(u (x) D), out = GLU(y^T W + b). Runs on
    all 8 cores as a Bass/Tile kernel, sharded over (batch, L) — fully
    data-parallel, no collectives.
  * slow path  (k != 0): exact host fallback with np.fft (never taken for
    the documented input distribution).
"""

import os

import numpy as np

import concourse.bass as bass
import concourse.mybir as mybir
from concourse import bacc
from concourse.bass_utils import run_bass_kernel_spmd
from concourse.tile import TileContext

# Problem dims (hardcoded per contract)
B, C, H, L = 4, 2, 768, 4096
KERNEL_LAM = 0.1
N_CORES = 8
P = 128

L_SH = (B * L) // N_CORES  # 2048 columns of L per core (half of one batch)
NSL = 512                  # matmul moving-operand free size (one PSUM bank)
N_LS = L_SH // NSL         # 4 l-slices per core
HT = H // P                # 6 h-tiles
FT = (C * H) // P          # 12 feature tiles (contraction dim)
CH = C * H                 # 1536
H2 = 2 * H                 # 1536 dense output cols

# matmul dtype: "bf16" (1 cyc/row) or "f32" (2 cyc/row)
MM_MODE = os.environ.get("LONGCONV_MM_DT", "bf16")


def _build_nc(mm_mode: str) -> bass.Bass:
    mm_dt = mybir.dt.bfloat16 if mm_mode == "bf16" else mybir.dt.float32
    f32 = mybir.dt.float32

    nc = bacc.Bacc(None, target_bir_lowering=False)
    u_d = nc.dram_tensor("u", [H, L_SH], f32, kind="ExternalInput")
    w_d = nc.dram_tensor("w", [CH, H2], mm_dt, kind="ExternalInput")
    d_d = nc.dram_tensor("dvec", [P, C * HT], f32, kind="ExternalInput")
    b_d = nc.dram_tensor("bvec", [P, FT], f32, kind="ExternalInput")
    o_d = nc.dram_tensor("out", [H, L_SH], f32, kind="ExternalOutput")

    gelu = mybir.ActivationFunctionType.Gelu_apprx_tanh
    sigm = mybir.ActivationFunctionType.Sigmoid

    with TileContext(nc) as tc:
        with (
            tc.tile_pool(name="consts", bufs=1) as cpool,
            tc.tile_pool(name="upool", bufs=2) as upool,
            tc.tile_pool(name="ypool", bufs=2) as ypool,
            tc.tile_pool(name="opool", bufs=3) as opool,
            tc.tile_pool(name="psum", bufs=4, space="PSUM") as pspool,
        ):
            d_t = cpool.tile([P, C * HT], f32)
            nc.sync.dma_start(out=d_t, in_=d_d[:, :])
            b_t = cpool.tile([P, FT], f32)
            nc.sync.dma_start(out=b_t, in_=b_d[:, :])
            # W resident: one big tile, column block ft holds W rows
            # [ft*128,(ft+1)*128) x [0,1536); single 3D-AP DMA
            w_t = cpool.tile([P, FT * H2], mm_dt)
            nc.sync.dma_start(
                out=w_t.rearrange("p (t n) -> p t n", t=FT),
                in_=w_d.rearrange("(t p) n -> p t n", p=P),
            )

            for ls in range(N_LS):
                lsl = slice(ls * NSL, (ls + 1) * NSL)
                # load u slice: 6 h-tiles of [128, NSL] in one 3D-AP DMA
                u_t = upool.tile([P, HT * NSL], f32)
                nc.sync.dma_start(
                    out=u_t.rearrange("p (t n) -> p t n", t=HT),
                    in_=u_d.rearrange("(t p) l -> p t l", p=P)[:, :, lsl],
                )
                # y[f, l] = gelu(D[f] * u[h, l]), f = c*H + h; 12 ACT ops
                y_t = ypool.tile([P, FT * NSL], mm_dt)
                for c in range(C):
                    for ht in range(HT):
                        ft = c * HT + ht
                        nc.scalar.activation(
                            y_t[:, ft * NSL : (ft + 1) * NSL],
                            u_t[:, ht * NSL : (ht + 1) * NSL],
                            gelu,
                            scale=d_t[:, ft : ft + 1],
                        )
                # dense + GLU, paired n-tiles (a: nt, g: nt+6)
                for nt in range(HT):
                    ps_a = pspool.tile([P, NSL], f32)
                    ps_g = pspool.tile([P, NSL], f32)
                    for ft in range(FT):
                        nc.tensor.matmul(
                            ps_a,
                            w_t[:, ft * H2 + nt * P : ft * H2 + (nt + 1) * P],
                            y_t[:, ft * NSL : (ft + 1) * NSL],
                            start=(ft == 0),
                            stop=(ft == FT - 1),
                        )
                    for ft in range(FT):
                        nc.tensor.matmul(
                            ps_g,
                            w_t[
                                :,
                                ft * H2 + (HT + nt) * P : ft * H2 + (HT + nt + 1) * P,
                            ],
                            y_t[:, ft * NSL : (ft + 1) * NSL],
                            start=(ft == 0),
                            stop=(ft == FT - 1),
                        )
                    # GLU: out = (a + b_a) * sigmoid(g + b_g)
                    sig_t = opool.tile([P, NSL], f32, tag="sig")
                    nc.scalar.activation(
                        sig_t, ps_g, sigm, bias=b_t[:, HT + nt : HT + nt + 1]
                    )
                    a_t = opool.tile([P, NSL], f32, tag="a")
                    nc.vector.tensor_scalar_add(a_t, ps_a, b_t[:, nt : nt + 1])
                    o_t = opool.tile([P, NSL], f32, tag="o")
                    nc.vector.tensor_mul(o_t, a_t, sig_t)
                    nc.sync.dma_start(
                        out=o_d[nt * P : (nt + 1) * P, lsl], in_=o_t
                    )
    nc.finalize()
    return nc


_NC_CACHE: dict[str, bass.Bass] = {}


def _get_nc(mm_mode: str) -> bass.Bass:
    if mm_mode not in _NC_CACHE:
        _NC_CACHE[mm_mode] = _build_nc(mm_mode)
    return _NC_CACHE[mm_mode]


def _make_in_maps(u, D, W, b, mm_mode: str) -> list[dict]:
    np_mm_dt = np.dtype("float32") if mm_mode == "f32" else mybir.dt.np(
        mybir.dt.bfloat16
    )
    w_host = np.ascontiguousarray(W).astype(np_mm_dt)
    d_host = np.ascontiguousarray(
        D.reshape(C, HT, P).transpose(2, 0, 1).reshape(P, C * HT)
    ).astype(np.float32)
    b_host = np.ascontiguousarray(b.reshape(FT, P).T).astype(np.float32)

    in_maps = []
    for core in range(N_CORES):
        bi, half = core // 2, core % 2
        u_s = np.ascontiguousarray(u[bi, :, half * L_SH : (half + 1) * L_SH])
        in_maps.append({"u": u_s, "w": w_host, "dvec": d_host, "bvec": b_host})
    return in_maps


def _fast_path(u, D, W, b, mm_mode: str) -> np.ndarray:
    nc = _get_nc(mm_mode)
    in_maps = _make_in_maps(u, D, W, b, mm_mode)
    res = run_bass_kernel_spmd(nc, in_maps, list(range(N_CORES)))
    out = np.empty((B, H, L), dtype=np.float32)
    for core in range(N_CORES):
        bi, half = core // 2, core % 2
        out[bi, :, half * L_SH : (half + 1) * L_SH] = res.results[core]["out"]
    return out


def _gelu_tanh(x):
    return 0.5 * x * (1.0 + np.tanh(np.sqrt(2.0 / np.pi) * (x + 0.044715 * x**3)))


def _slow_path(u, D, kernel, W, b) -> np.ndarray:
    """Exact host fallback (never taken for the documented input dist)."""
    n = 2 * L
    k = np.maximum(np.abs(kernel) - KERNEL_LAM, 0.0) * np.sign(kernel)
    k_f = np.fft.rfft(k.astype(np.float64), n=n)
    u_f = np.fft.rfft(u.astype(np.float64), n=n)
    y_f = np.einsum("bhl,chl->bchl", u_f, k_f)
    y = np.fft.irfft(y_f, n=n)[..., :L]
    y = y + np.einsum("bhl,ch->bchl", u.astype(np.float64), D.astype(np.float64))
    y = y.reshape(B, C * H, L)
    y = _gelu_tanh(y)
    y = y.transpose(0, 2, 1) @ W.astype(np.float64) + b.astype(np.float64)
    y = y[..., :H] * (1.0 / (1.0 + np.exp(-y[..., H:])))
    return y.transpose(0, 2, 1).astype(np.float32)


def kernel(u, D, kernel, W, b) -> np.ndarray:
    u = np.asarray(u, dtype=np.float32)
    D = np.asarray(D, dtype=np.float32)
    kernel = np.asarray(kernel, dtype=np.float32)
    W = np.asarray(W, dtype=np.float32)
    b = np.asarray(b, dtype=np.float32)

    # Exact check on the actual data: soft-threshold zeroes the conv kernel
    # iff every |kernel| <= lam. True w.p. ~1 for kernel ~ 0.002*randn.
    if float(np.abs(kernel).max()) <= KERNEL_LAM:
        return _fast_path(u, D, W, b, MM_MODE)
    return _slow_path(u, D, kernel, W, b)


# revision 8
# speedup vs baseline: 1.0525x; 1.0525x over previous
"""LongConv kernel for Trainium2 (8 NeuronCores, SPMD).

Reference computation (B=4, C=2, H=768, L=4096):
    k   = soft_threshold(kernel, lam=0.1)            # (C, H, 2L)
    y   = irfft(rfft(u, 2L) * rfft(k, 2L))[..., :L]  # FFT long conv
    y  += u * D                                      # skip
    y   = gelu(y.reshape(B, C*H, L))                 # tanh-approx gelu
    out = GLU((y^T @ W + b))^T                       # (B, H, L)

Key algebraic fact: kernel is drawn as 0.002*randn and lam=0.1, so the
soft-threshold zeroes it out exactly (would need a 50-sigma sample not to).
We verify that on the actual data (exact elementwise check, not an
assumption) and dispatch:

  * fast path  (k == 0): y = gelu(u (x) D), out = GLU(y^T W + b). Runs on
    all 8 cores as a Bass/Tile kernel, sharded over (batch, L) — fully
    data-parallel, no collectives.
  * slow path  (k != 0): exact host fallback with np.fft (never taken for
    the documented input distribution).

Fast-path kernel design notes:
  * Dense is W-stationary on PE: out[n, l] = sum_f W[f, n] * y[f, l], so the
    output lands directly in (feature, length) layout — no transposes.
  * gelu is computed as 0.5*x*(1 + erf(x/sqrt(2))): Erf and Sigmoid (for the
    GLU gate) live in the SAME ACT table set ("sigmoid_and_others"), so the
    scalar engine never reloads activation tables (a reload costs ~4.4us and
    an interleaved gelu/sigmoid stream thrashes it). The erf-gelu matches the
    reference tanh-gelu to ~3e-4*x^4 (|x| <~ 0.3 here since D ~ 0.01*randn).
  * W is host-relaid into per-(n-tile-pair) contiguous chunks so the first
    matmul group only waits on a ~0.8 MB DMA, not the full 4.7 MB.
"""

import os

import numpy as np

import concourse.bass as bass
import concourse.mybir as mybir
from concourse import bacc
from concourse.bass_utils import run_bass_kernel_spmd
from concourse.tile import TileContext

# Problem dims (hardcoded per contract)
B, C, H, L = 4, 2, 768, 4096
KERNEL_LAM = 0.1
N_CORES = 8
P = 128

L_SH = (B * L) // N_CORES  # 2048 columns of L per core (half of one batch)
NSL = 512                  # matmul moving-operand free size (one PSUM bank)
N_LS = L_SH // NSL         # 4 l-slices per core
HT = H // P                # 6 h-tiles (and n-tile pairs)
FT = (C * H) // P          # 12 feature tiles (contraction dim)
CH = C * H                 # 1536
H2 = 2 * H                 # 1536 dense output cols
WCH = FT * 2 * P           # 3072 cols per W chunk (12 ft x [a|g] x 128)

# matmul dtype: "bf16" (1 cyc/row) or "f32" (2 cyc/row)
MM_MODE = os.environ.get("LONGCONV_MM_DT", "bf16")


def _build_nc(mm_mode: str, has_bias: bool) -> bass.Bass:
    mm_dt = mybir.dt.bfloat16 if mm_mode == "bf16" else mybir.dt.float32
    f32 = mybir.dt.float32

    nc = bacc.Bacc(None, target_bir_lowering=False)
    u_d = nc.dram_tensor("u", [H, L_SH], f32, kind="ExternalInput")
    w_d = nc.dram_tensor("w", [P, HT * WCH], mm_dt, kind="ExternalInput")
    d_d = nc.dram_tensor("dvec", [P, 2 * FT], f32, kind="ExternalInput")
    if has_bias:
        b_d = nc.dram_tensor("bvec", [P, FT], f32, kind="ExternalInput")
    o_d = nc.dram_tensor("out", [H, L_SH], f32, kind="ExternalOutput")

    erf = mybir.ActivationFunctionType.Erf
    sigm = mybir.ActivationFunctionType.Sigmoid

    with TileContext(nc) as tc:
        with (
            tc.tile_pool(name="consts", bufs=1) as cpool,
            tc.tile_pool(name="upool", bufs=2) as upool,
            tc.tile_pool(name="ypool", bufs=3) as ypool,
            tc.tile_pool(name="epool", bufs=4) as epool,
            tc.tile_pool(name="opool", bufs=4) as opool,
            tc.tile_pool(name="psa", bufs=3, space="PSUM") as psa_pool,
            tc.tile_pool(name="psg", bufs=3, space="PSUM") as psg_pool,
        ):
            d_t = cpool.tile([P, 2 * FT], f32)
            nc.sync.dma_start(out=d_t, in_=d_d[:, :])
            if has_bias:
                b_t = cpool.tile([P, FT], f32)
                nc.sync.dma_start(out=b_t, in_=b_d[:, :])

            # W resident, chunked per n-tile pair (contiguous in DRAM after
            # host re-layout); chunk 0 first so PE can start early.
            w_t = cpool.tile([P, HT * WCH], mm_dt)
            nc.sync.dma_start(
                out=w_t[:, 0:WCH], in_=w_d[:, 0:WCH]
            )
            # u slice 0 next, then the rest of W
            u_ts = []
            u_re = u_d.rearrange("(t p) l -> p t l", p=P)
            u0 = upool.tile([P, HT * NSL], f32, tag="u")
            nc.sync.dma_start(
                out=u0.rearrange("p (t n) -> p t n", t=HT),
                in_=u_re[:, :, 0:NSL],
            )
            u_ts.append(u0)
            for np_ in range(1, HT):
                nc.sync.dma_start(
                    out=w_t[:, np_ * WCH : (np_ + 1) * WCH],
                    in_=w_d[:, np_ * WCH : (np_ + 1) * WCH],
                )
            for ls in range(1, N_LS):
                ut = upool.tile([P, HT * NSL], f32, tag="u")
                nc.sync.dma_start(
                    out=ut.rearrange("p (t n) -> p t n", t=HT),
                    in_=u_re[:, :, ls * NSL : (ls + 1) * NSL],
                )
                u_ts.append(ut)

            for ls in range(N_LS):
                lsl = slice(ls * NSL, (ls + 1) * NSL)
                u_t = u_ts[ls]
                # y[f, l] = gelu(D[f] * u[h, l]) via erf:
                #   e  = erf((D/sqrt(2)) * u)      [ACT, sigmoid table set]
                #   x2 = (D/2) * u                 [DVE]
                #   y  = x2 * (1 + e)              [DVE x2, bf16 out]
                y_t = ypool.tile([P, FT * NSL], mm_dt)
                for c in range(C):
                    for ht in range(HT):
                        ft = c * HT + ht
                        usl = u_t[:, ht * NSL : (ht + 1) * NSL]
                        e_t = epool.tile([P, NSL], f32, tag="e")
                        nc.scalar.activation(
                            e_t, usl, erf, scale=d_t[:, ft : ft + 1]
                        )
                        x2 = epool.tile([P, NSL], f32, tag="x2")
                        nc.vector.tensor_scalar_mul(
                            x2, usl, d_t[:, FT + ft : FT + ft + 1]
                        )
                        nc.vector.tensor_scalar_add(e_t, e_t, 1.0)
                        nc.vector.tensor_mul(
                            y_t[:, ft * NSL : (ft + 1) * NSL], x2, e_t
                        )
                # dense + GLU, paired n-tiles (a: nt, g: nt+6)
                for np_ in range(HT):
                    ps_a = psa_pool.tile([P, NSL], f32)
                    ps_g = psg_pool.tile([P, NSL], f32)
                    wbase = np_ * WCH
                    for ft in range(FT):
                        nc.tensor.matmul(
                            ps_a,
                            w_t[:, wbase + ft * 256 : wbase + ft * 256 + 128],
                            y_t[:, ft * NSL : (ft + 1) * NSL],
                            start=(ft == 0),
                            stop=(ft == FT - 1),
                        )
                    for ft in range(FT):
                        nc.tensor.matmul(
                            ps_g,
                            w_t[
                                :,
                                wbase + ft * 256 + 128 : wbase + (ft + 1) * 256,
                            ],
                            y_t[:, ft * NSL : (ft + 1) * NSL],
                            start=(ft == 0),
                            stop=(ft == FT - 1),
                        )
                    # GLU: out = (a + b_a) * sigmoid(g + b_g)
                    sig_t = opool.tile([P, NSL], f32, tag="sig")
                    if has_bias:
                        nc.scalar.activation(
                            sig_t, ps_g, sigm,
                            bias=b_t[:, HT + np_ : HT + np_ + 1],
                        )
                        a_t = opool.tile([P, NSL], f32, tag="a")
                        nc.vector.tensor_scalar_add(
                            a_t, ps_a, b_t[:, np_ : np_ + 1]
                        )
                    else:
                        nc.scalar.activation(sig_t, ps_g, sigm)
                        a_t = ps_a
                    o_t = opool.tile([P, NSL], f32, tag="o")
                    nc.vector.tensor_mul(o_t, a_t, sig_t)
                    nc.sync.dma_start(
                        out=o_d[np_ * P : (np_ + 1) * P, lsl], in_=o_t
                    )
    nc.finalize()
    return nc


_NC_CACHE: dict[tuple, bass.Bass] = {}


def _get_nc(mm_mode: str, has_bias: bool) -> bass.Bass:
    key = (mm_mode, has_bias)
    if key not in _NC_CACHE:
        _NC_CACHE[key] = _build_nc(mm_mode, has_bias)
    return _NC_CACHE[key]


def _make_in_maps(u, D, W, b, mm_mode: str, has_bias: bool) -> list[dict]:
    np_mm_dt = np.dtype("float32") if mm_mode == "f32" else mybir.dt.np(
        mybir.dt.bfloat16
    )
    # per-npair contiguous W chunks:
    # w_re[p, np_, ft, half*128 + j] = W[ft*128 + p, half*768 + np_*128 + j]
    w_re = np.ascontiguousarray(
        W.reshape(FT, P, 2, HT, P).transpose(1, 3, 0, 2, 4).reshape(P, HT * WCH)
    ).astype(np_mm_dt)
    # d cols [0, FT): D/sqrt(2) (erf scale); [FT, 2FT): D/2
    d_pf = D.reshape(C, HT, P).transpose(2, 0, 1).reshape(P, FT)
    d_host = np.ascontiguousarray(
        np.concatenate([d_pf / np.sqrt(2.0), d_pf * 0.5], axis=1)
    ).astype(np.float32)
    b_host = np.ascontiguousarray(b.reshape(FT, P).T).astype(np.float32)

    in_maps = []
    for core in range(N_CORES):
        bi, half = core // 2, core % 2
        u_s = np.ascontiguousarray(u[bi, :, half * L_SH : (half + 1) * L_SH])
        m = {"u": u_s, "w": w_re, "dvec": d_host}
        if has_bias:
            m["bvec"] = b_host
        in_maps.append(m)
    return in_maps


def _fast_path(u, D, W, b, mm_mode: str) -> np.ndarray:
    has_bias = bool(np.any(b))
    nc = _get_nc(mm_mode, has_bias)
    in_maps = _make_in_maps(u, D, W, b, mm_mode, has_bias)
    res = run_bass_kernel_spmd(nc, in_maps, list(range(N_CORES)))
    out = np.empty((B, H, L), dtype=np.float32)
    for core in range(N_CORES):
        bi, half = core // 2, core % 2
        out[bi, :, half * L_SH : (half + 1) * L_SH] = res.results[core]["out"]
    return out


def _gelu_tanh(x):
    return 0.5 * x * (1.0 + np.tanh(np.sqrt(2.0 / np.pi) * (x + 0.044715 * x**3)))


def _slow_path(u, D, kernel, W, b) -> np.ndarray:
    """Exact host fallback (never taken for the documented input dist)."""
    n = 2 * L
    k = np.maximum(np.abs(kernel) - KERNEL_LAM, 0.0) * np.sign(kernel)
    k_f = np.fft.rfft(k.astype(np.float64), n=n)
    u_f = np.fft.rfft(u.astype(np.float64), n=n)
    y_f = np.einsum("bhl,chl->bchl", u_f, k_f)
    y = np.fft.irfft(y_f, n=n)[..., :L]
    y = y + np.einsum("bhl,ch->bchl", u.astype(np.float64), D.astype(np.float64))
    y = y.reshape(B, C * H, L)
    y = _gelu_tanh(y)
    y = y.transpose(0, 2, 1) @ W.astype(np.float64) + b.astype(np.float64)
    y = y[..., :H] * (1.0 / (1.0 + np.exp(-y[..., H:])))
    return y.transpose(0, 2, 1).astype(np.float32)


def kernel(u, D, kernel, W, b) -> np.ndarray:
    u = np.asarray(u, dtype=np.float32)
    D = np.asarray(D, dtype=np.float32)
    kernel = np.asarray(kernel, dtype=np.float32)
    W = np.asarray(W, dtype=np.float32)
    b = np.asarray(b, dtype=np.float32)

    # Exact check on the actual data: soft-threshold zeroes the conv kernel
    # iff every |kernel| <= lam. True w.p. ~1 for kernel ~ 0.002*randn.
    if float(np.abs(kernel).max()) <= KERNEL_LAM:
        return _fast_path(u, D, W, b, MM_MODE)
    return _slow_path(u, D, kernel, W, b)
